# revision 2
# baseline (speedup 1.0000x reference)
"""DropConnect forward kernel v3 for Trainium2 (8 NeuronCores, Bass/Tile).

y[n,o] = (sum_k x[n,k] * weight[k,o] * w_mask[n,k,o] + bias[o]*b_mask[n,o]) * 2

Two slab kinds per core, balancing the DMA roofline (~358 GB/s/core)
against the DVE roofline (~245 Gelem/s bf16 tensor_tensor):

- PACKED pair-slab (NPK pairs): c = m_even + 2*m_odd - 1.5, bf16, 1 MiB
  per sample of DMA but two DVE TTs + one ACT sign per pair:
    ACT: s = sign(c) = 2*m1 - 1;  DVE: Q = c*w2, R = s*w2
    PE rows: 2i   += x0*Q - x0*R + x0/2*w2   = x0*(m0*w2)
             2i+1 += x1/2*R + x1/2*w2        = x1*(m1*w2)
- PREMULT sample-slab (NPRE samples): host sends A = m (*) w2 directly
  (exact in bf16), 2 MiB per sample of DMA but zero DVE/ACT work: the
  PE streams the slab straight from SBUF: row n += x_n * A.

All chains accumulate into one PSUM region whose 32 rows are the 32
samples; eviction is two batched [32,512] copies + the bias epilogue.
The w2-const chain runs first (warms the PE; gated only by the tiny xw
DMA on the sync ring). The first packed slab is split in halves so the
DVE starts ~5 us earlier. The last slabs are premult so the DVE tail
drains before the DMA stream ends.
"""

import sys

for _p in ("/opt/trn_rl_repo",):
    if _p not in sys.path:
        sys.path.insert(0, _p)

import numpy as np

import concourse.bass as bass
import concourse.tile as tile
from concourse import bacc, mybir
from concourse.bass_utils import run_bass_kernel_spmd

N_CORES = 8
NS = 32            # samples per core
NPRE = 6           # premultiplied sample-slabs per core (even)
NPK = (NS - NPRE) // 2  # packed pair-slabs per core
D = 1024
P = 128
J = D // P         # k = 8p + j
F = J * D          # 8192 free elements per slab
H = F // 2
NH = 512           # one fp32 PSUM bank width

FP32 = mybir.dt.float32
BF16 = mybir.dt.bfloat16

TRACE = {"trace": False, "last_result": None, "trace_kwargs": {}}


NGE = 0  # GPSIMD extraction disabled: Q7 SBUF-port contention with DVE 2-port TTs is catastrophic (63us/op measured)


def _ge_pairs(npk):
    return set(range(max(0, npk - NGE), npk))


def _slab_order(npk, npre):
    """First few slabs packed (DVE ramps immediately), then a Bresenham
    merge, ending premult (DVE tail drains before the DMA stream ends)."""
    lead = min(3, npk)
    order = [("pk", i) for i in range(lead)]
    pi, ai = lead, 0
    rem_pk = npk - lead
    while pi < npk or ai < npre:
        if pi < npk and (ai >= npre or (pi - lead) * (npre + 1) <= ai * (rem_pk + 1)):
            order.append(("pk", pi))
            pi += 1
        else:
            order.append(("pre", ai))
            ai += 1
    return order


def _build_nc(npk: int = NPK, npre: int = NPRE):
    ns = 2 * npk + npre
    SW = ns
    nslab = npk + npre
    order = _slab_order(npk, npre)
    ge_pairs = _ge_pairs(npk)

    nc = bacc.Bacc("TRN2", target_bir_lowering=False, debug=False)

    wm = nc.declare_dram_parameter("wm", [nslab, P, F], BF16, isOutput=False)
    w2 = nc.declare_dram_parameter("w2", [P, F], BF16, isOutput=False)
    sq = nc.declare_dram_parameter("sq", [P, nslab * J * SW], BF16, isOutput=False)
    sr = nc.declare_dram_parameter("sr", [P, max(npk, 1) * J * SW], BF16, isOutput=False)
    xw = nc.declare_dram_parameter("xw", [P, J * SW], BF16, isOutput=False)
    bb = nc.declare_dram_parameter("bb", [ns, D], FP32, isOutput=False)
    y = nc.declare_dram_parameter("y", [ns, D], FP32, isOutput=True)

    with tile.TileContext(nc) as tc:
        with (
            tc.tile_pool(name="const", bufs=1) as cpool,
            tc.tile_pool(name="pk", bufs=6) as spool,
            tc.tile_pool(name="pre", bufs=2) as prepool,
            tc.tile_pool(name="sg", bufs=2) as gpool,
            tc.tile_pool(name="q", bufs=4) as qpool,
            tc.tile_pool(name="r", bufs=4) as rpool,
            tc.tile_pool(name="psum", bufs=1, space=bass.MemorySpace.PSUM) as ppool,
        ):
            # Scalar ring: w2 halves first (gate DVE), then sq/sr chunks
            # (gate PE per-slab chains), bb last. Sync ring: xw (tiny,
            # gates the PE-warming w2-chain) ahead of the mask slabs.
            w2t = cpool.tile([P, F], BF16, tag="w2")
            nc.scalar.dma_start(out=w2t[:, 0:H], in_=w2[:, 0:H])
            nc.scalar.dma_start(out=w2t[:, H:F], in_=w2[:, H:F])
            sqt = cpool.tile([P, nslab * J * SW], BF16, tag="sq")
            srt = cpool.tile([P, max(npk, 1) * J * SW], BF16, tag="sr")
            BW = J * SW  # one slab-block of stationary columns
            qbounds = [bb_ * BW for bb_ in range(0, nslab, max(nslab // 4, 1))] + [nslab * BW]
            rbounds = [bb_ * BW for bb_ in range(0, max(npk, 1), max(npk // 3, 1))] + [max(npk, 1) * BW]
            for cc in range(max(len(qbounds), len(rbounds)) - 1):
                if cc < len(qbounds) - 1:
                    a, b = qbounds[cc], qbounds[cc + 1]
                    nc.scalar.dma_start(out=sqt[:, a:b], in_=sq[:, a:b])
                if cc < len(rbounds) - 1:
                    a, b = rbounds[cc], rbounds[cc + 1]
                    nc.scalar.dma_start(out=srt[:, a:b], in_=sr[:, a:b])
            bbt = cpool.tile([ns, D], FP32, tag="bb")
            nc.scalar.dma_start(out=bbt[:], in_=bb[:])
            xwt = cpool.tile([P, J * SW], BF16, tag="xw")
            nc.sync.dma_start(out=xwt[:], in_=xw[:])
            stage = cpool.tile([ns, D], FP32, tag="stage")
            yt = cpool.tile([ns, D], FP32, tag="y")

            ps = ppool.tile([32 + SW, NH], FP32, tag="ps")

            # w2-const chain first: starts the psum group, warms the PE.
            for j in range(J):
                for g in range(2):
                    nc.tensor.matmul(
                        ps[32 * g : 32 * g + SW, :],
                        xwt[:, j * SW : (j + 1) * SW],
                        w2t[:, j * D + NH * g : j * D + NH * g + NH],
                        start=(j == 0),
                        stop=False,
                        tile_position=(0, 32 * g),
                        skip_group_check=True,
                    )

            n_mm = [0, 0]
            per_group = nslab * J + npk * J  # MMs per col-group

            def mm(lhs_slice, rhs, g):
                n_mm[g] += 1
                nc.tensor.matmul(
                    ps[32 * g : 32 * g + SW, :],
                    lhs_slice,
                    rhs,
                    start=False,
                    stop=(n_mm[g] == per_group),
                    tile_position=(0, 32 * g),
                    skip_group_check=True,
                )

            for t, (kind, idx) in enumerate(order):
                if kind == "pre":
                    # host-premultiplied m(*)w2: PE streams it directly
                    slab = prepool.tile([P, F], BF16, tag="pre")
                    nc.sync.dma_start(out=slab[:], in_=wm[t, :, :])
                    for j in range(J):
                        for g in range(2):
                            mm(
                                sqt[:, (t * J + j) * SW : (t * J + j + 1) * SW],
                                slab[:, j * D + NH * g : j * D + NH * g + NH],
                                g,
                            )
                    continue

                for hh in range(2):
                    fa = hh * H
                    slab = spool.tile([P, H], BF16, tag="pk")
                    nc.sync.dma_start(out=slab[:], in_=wm[t, :, fa : fa + H])
                    sg = gpool.tile([P, H], BF16, tag="sg")
                    if idx in ge_pairs:
                        # m1 = [c' >= 0] on the otherwise-idle GPSIMD
                        nc.gpsimd.tensor_scalar(
                            sg[:], slab[:], 0.0, None, op0=mybir.AluOpType.is_ge
                        )
                    else:
                        nc.scalar.sign(sg[:], slab[:])
                    q = qpool.tile([P, H], BF16, tag="q")
                    nc.vector.tensor_mul(q[:], slab[:], w2t[:, fa : fa + H])
                    r = rpool.tile([P, H], BF16, tag="r")
                    nc.vector.tensor_mul(r[:], sg[:], w2t[:, fa : fa + H])

                    for jj in range(J // 2):
                        j = hh * (J // 2) + jj
                        for g in range(2):
                            o0 = jj * D + NH * g
                            mm(
                                sqt[:, (t * J + j) * SW : (t * J + j + 1) * SW],
                                q[:, o0 : o0 + NH],
                                g,
                            )
                            mm(
                                srt[:, (idx * J + j) * SW : (idx * J + j + 1) * SW],
                                r[:, o0 : o0 + NH],
                                g,
                            )

            assert n_mm == [per_group, per_group], (n_mm, per_group)

            nc.scalar.copy(stage[0:SW, 0:NH], ps[0:SW, :])
            nc.scalar.copy(stage[0:SW, NH:D], ps[32 : 32 + SW, :])
            nc.vector.tensor_add(yt[:], stage[:], bbt[:])
            nc.scalar.dma_start(out=y[:], in_=yt[:])

    nc.compile()
    return nc


def _prep_core(xs, w2f, bias2, bms, npk, npre):
    """Lay out one core's tensors. xs: [ns, D] f32, w2f: [D, D]*2 f32,
    bias2: [D] (2*bias), bms: [ns, D] b_mask."""
    import ml_dtypes

    ns = 2 * npk + npre
    order = _slab_order(npk, npre)
    ge_pairs = _ge_pairs(npk)
    w2s = w2f.reshape(P, F)                      # k = 8p + j
    xt = np.ascontiguousarray(xs.T.reshape(P, J, ns))  # x[n, 8p+j] at [p, j, n]

    wm = np.empty((len(order), P, F), dtype=np.float32)
    sq = np.zeros((len(order), P, J, ns), dtype=np.float32)
    sr = np.zeros((max(npk, 1), P, J, ns), dtype=np.float32)
    xw = np.zeros((P, J, ns), dtype=np.float32)

    for t, (kind, idx) in enumerate(order):
        if kind == "pk":
            n0, n1 = 2 * idx, 2 * idx + 1
            wm[t] = (
                _prep_core.masks[n0] + 2.0 * _prep_core.masks[n1] - 1.5
            ).reshape(P, F)
            sq[t, :, :, n0] = xt[:, :, n0]
            if idx in ge_pairs:
                sr[idx, :, :, n0] = -2.0 * xt[:, :, n0]
                sr[idx, :, :, n1] = xt[:, :, n1]
                xw[:, :, n0] = 1.5 * xt[:, :, n0]
            else:
                sr[idx, :, :, n0] = -xt[:, :, n0]
                sr[idx, :, :, n1] = 0.5 * xt[:, :, n1]
                xw[:, :, n0] = 0.5 * xt[:, :, n0]
                xw[:, :, n1] = 0.5 * xt[:, :, n1]
        else:
            n = 2 * npk + idx
            wm[t] = (_prep_core.masks[n] * w2s.astype(np.float32)).reshape(P, F)
            sq[t, :, :, n] = xt[:, :, n]

    return {
        "wm": wm.astype(ml_dtypes.bfloat16),
        "w2": w2s.astype(ml_dtypes.bfloat16),
        "sq": np.ascontiguousarray(
            sq.transpose(1, 0, 2, 3).reshape(P, -1)
        ).astype(ml_dtypes.bfloat16),
        "sr": np.ascontiguousarray(
            sr.transpose(1, 0, 2, 3).reshape(P, -1)
        ).astype(ml_dtypes.bfloat16),
        "xw": xw.reshape(P, -1).astype(ml_dtypes.bfloat16),
        "bb": np.ascontiguousarray(bias2[None, :] * bms, dtype=np.float32),
    }


def _host_prep(x, weight, bias, w_mask, b_mask):
    import ml_dtypes

    x = np.ascontiguousarray(x, dtype=np.float32)
    # round w2 to bf16 once so premult slabs and the packed path share
    # the exact same effective weights
    w2bf = (2.0 * np.float32(weight)).astype(ml_dtypes.bfloat16)
    w2f = w2bf.astype(np.float32)
    bias2 = 2.0 * np.float32(bias)
    b_mask = np.ascontiguousarray(b_mask, dtype=np.float32)

    in_maps = []
    for c in range(N_CORES):
        sl = slice(c * NS, (c + 1) * NS)
        _prep_core.masks = np.asarray(w_mask[sl], dtype=np.float32).reshape(
            NS, P, F
        )
        in_maps.append(
            _prep_core(x[sl], w2f, bias2, b_mask[sl], NPK, NPRE)
        )
    return in_maps


def kernel(x, weight, bias, w_mask, b_mask):
    x, weight, bias, w_mask, b_mask = (
        np.asarray(a) for a in (x, weight, bias, w_mask, b_mask)
    )
    in_maps = _host_prep(x, weight, bias, w_mask, b_mask)
    nc = _build_nc()
    res = run_bass_kernel_spmd(
        nc,
        in_maps,
        core_ids=list(range(N_CORES)),
        trace=TRACE["trace"],
        **TRACE["trace_kwargs"],
    )
    TRACE["last_result"] = res
    out = np.concatenate([res.results[c]["y"] for c in range(N_CORES)], axis=0)
    return out.astype(np.float32, copy=False)
